# revision 1
# baseline (speedup 1.0000x reference)
"""ActivateAttention Trainium2 kernel — 8 NeuronCores, SPMD, no collectives.

Sharding: core i handles batch b=i//4 and query-row quarter qc=i%4
(1024 query rows), all 12 heads, full K/V for its batch. Each core's
output rows are complete, so the host just concatenates — no all-reduce.

Per-core pipeline (bf16 compute, f32 PSUM accumulate):
  1. weights:  DMA f32 -> cast bf16 -> PE-transpose -> W^T in SBUF
  2. inputs:   DMA f32 tiles -> cast bf16 -> PE-transpose -> x^T blocks
  3. proj:     q^T/k^T = W^T.T @ x^T  (k: +bias, exact GELU on ACT);
               v natural [n,do] with a ones column appended per head
  4. attn:     S^T chunks [128,1536] in PSUM = K^T.T @ Q^T (K=64 contraction);
               ACT exp(SCALE*S) PSUM->SBUF bf16 (no max-subtraction; logits
               are ~N(0,<1) so exp is safe); PV accumulates x_aug[q,65]
               per 8-k-tile segment (ones column gives softmax denominators)
  5. norm+out: x * recip(denominator) -> transpose -> out = x^T.T @ Wp^T + bp
"""

import numpy as np
from contextlib import ExitStack

from concourse import bass, bacc, mybir, masks, tile
from concourse import bass_utils

F32 = mybir.dt.float32
BF16 = mybir.dt.bfloat16
AF = mybir.ActivationFunctionType
ALU = mybir.AluOpType

B = 2
N = 4096
DIM = 768
H = 12
D = 64
SCALE = D ** -0.5          # 1/8
NQ = 1024                  # query rows per core
N_CORES = 8

NT_Q = NQ // 128           # 8 query tiles per core
NT_K = N // 128            # 32 key tiles
NCT = DIM // 128           # 6 channel tiles
CHUNK = 1536               # S^T exp chunk (3 PSUM banks)
SEG = 8                    # k-tiles per PV psum accumulation segment


def build_nc() -> bass.Bass:
    nc = bacc.Bacc("TRN2", target_bir_lowering=False, debug=False)

    query = nc.declare_dram_parameter("query", [NQ, DIM], F32, False).ap()
    key = nc.declare_dram_parameter("key", [N, DIM], F32, False).ap()
    value = nc.declare_dram_parameter("value", [N, DIM], F32, False).ap()
    Wq = nc.declare_dram_parameter("Wq", [DIM, DIM], F32, False).ap()
    Wk = nc.declare_dram_parameter("Wk", [DIM, DIM], F32, False).ap()
    bk = nc.declare_dram_parameter("bk", [DIM], F32, False).ap()
    Wv = nc.declare_dram_parameter("Wv", [DIM, DIM], F32, False).ap()
    Wp = nc.declare_dram_parameter("Wp", [DIM, DIM], F32, False).ap()
    bp = nc.declare_dram_parameter("bp", [DIM], F32, False).ap()
    out = nc.declare_dram_parameter("out", [NQ, DIM], F32, True).ap()

    with tile.TileContext(nc) as tc, ExitStack() as ctx:
        # ---------------- persistent pools ----------------
        const_pool = ctx.enter_context(tc.tile_pool(name="const", bufs=1))
        ident = const_pool.tile([128, 128], BF16)
        masks.make_identity(nc, ident[:])

        # bk as per-partition bias columns: bk_t[p, m] = bk[128*m + p]
        bk_t = const_pool.tile([128, NCT], F32)
        nc.sync.dma_start(out=bk_t[:], in_=bk.rearrange("(c p) -> p c", p=128))
        bp_row = const_pool.tile([1, DIM], F32)
        nc.sync.dma_start(out=bp_row[:], in_=bp.rearrange("(a c) -> a c", a=1))
        # broadcast bp across partitions via a [1,128]-ones matmul
        ones_col = const_pool.tile([1, 128], F32)
        nc.vector.memset(ones_col[:], 1.0)
        bp_bcast = const_pool.tile([128, DIM], F32)
        with tc.tile_pool(name="bpp", bufs=1, space="PSUM") as bpp:
            for j in range(2):
                sl = slice(512 * j, 512 * (j + 1)) if j == 0 else slice(512, DIM)
                pt = bpp.tile([128, 512 if j == 0 else DIM - 512], F32)
                nc.tensor.matmul(pt[:], ones_col[:], bp_row[:, sl],
                                 start=True, stop=True)
                nc.vector.tensor_copy(bp_bcast[:, sl], pt[:])

        # transposed weights:
        #   wqk_t: [Wq^T | Wk^T] as lhsT chunks   [128, c-tile, 1536]
        #   wv_t / wp_t: rhs (moving) layout      [128, c-tile, 768]
        wp_t = const_pool.tile([128, NCT, DIM], BF16)

        # persistent projected tensors
        qT = [const_pool.tile([128, NQ], BF16, name=f"qT{m}", tag=f"qT{m}")
              for m in range(NCT)]                       # q^T  [do, n] per m
        kT = [[const_pool.tile([128, 1024], BF16, name=f"kT{m}_{j}",
                               tag=f"kT{m}_{j}") for j in range(N // 1024)]
              for m in range(NCT)]                       # gelu(k^T + bk)
        v_aug = [const_pool.tile([128, H * 65], BF16, name=f"vaug{t}",
                                 tag=f"vaug{t}")
                 for t in range(NT_K)]
        xT = const_pool.tile([128, NCT, NQ], BF16)       # attn out transposed

        wvpool_cm = tc.tile_pool(name="wvpool", bufs=1)
        wvpool = wvpool_cm.__enter__()
        wv_t = wvpool.tile([128, NCT, DIM], BF16)
        wpool_cm = tc.tile_pool(name="wpool", bufs=1)
        wpool = wpool_cm.__enter__()
        wqk_t = wpool.tile([128, NCT, 2 * DIM], BF16)

        with tc.tile_pool(name="wstage", bufs=3) as wstage, \
             tc.tile_pool(name="wpsum", bufs=3, space="PSUM") as wpsum:
            wlist = [(Wq, wqk_t, 0), (Wk, wqk_t, DIM), (Wv, wv_t, 0)]
            for wsrc, wdst, col0 in wlist:
                for r in range(NCT):          # 128-row tile of W [do, c]
                    wf = wstage.tile([128, DIM], F32, tag="wf")
                    nc.sync.dma_start(out=wf[:], in_=wsrc[128 * r:128 * (r + 1), :])
                    wb = wstage.tile([128, DIM], BF16, tag="wb")
                    nc.vector.tensor_copy(wb[:], wf[:])
                    for c in range(NCT):
                        tp = wpsum.tile([128, 128], BF16)
                        nc.tensor.transpose(tp[:], wb[:, 128 * c:128 * (c + 1)],
                                            ident[:])
                        nc.vector.tensor_copy(
                            wdst[:, c, col0 + 128 * r: col0 + 128 * (r + 1)], tp[:])

        # ---------------- input load + transpose + projection ----------------
        # 1024-col n-chunks; psum: 3 m-tiles x [128,1024] (6 banks) + tp (2)
        def project_qk(src_ap, n_rows, wcol0, dst, gelu):
            nchunks = n_rows // 1024
            with tc.tile_pool(name="ldstage", bufs=4) as ldstage, \
                 tc.tile_pool(name="tpsum", bufs=2, space="PSUM") as tpsum, \
                 tc.tile_pool(name="xtb", bufs=2) as xtb, \
                 tc.tile_pool(name="mpsum", bufs=3, space="PSUM") as mpsum:
                for j in range(nchunks):
                    xt = xtb.tile([128, NCT, 1024], BF16, tag="xt")
                    for t in range(8):        # eight 128-row tiles
                        row0 = 1024 * j + 128 * t
                        xf = ldstage.tile([128, DIM], F32, tag="xf")
                        nc.sync.dma_start(out=xf[:], in_=src_ap[row0:row0 + 128, :])
                        xb = ldstage.tile([128, DIM], BF16, tag="xb")
                        nc.vector.tensor_copy(xb[:], xf[:])
                        tp = tpsum.tile([128, NCT, 128], BF16)
                        for c in range(NCT):
                            nc.tensor.transpose(tp[:, c, :],
                                                xb[:, 128 * c:128 * (c + 1)],
                                                ident[:])
                        nc.vector.tensor_copy(
                            xt[:, :, 128 * t:128 * (t + 1)], tp[:])
                    for m in range(NCT):
                        pm = mpsum.tile([128, 1024], F32)
                        for c in range(NCT):
                            for qj in range(2):
                                nc.tensor.matmul(
                                    pm[:, 512 * qj:512 * (qj + 1)],
                                    wqk_t[:, c,
                                          wcol0 + 128 * m: wcol0 + 128 * (m + 1)],
                                    xt[:, c, 512 * qj:512 * (qj + 1)],
                                    start=(c == 0), stop=(c == NCT - 1))
                        dsl = dst(m, j)
                        if gelu:
                            nc.scalar.activation(dsl, pm[:], AF.Gelu,
                                                 bias=bk_t[:, m:m + 1], scale=1.0)
                        else:
                            nc.vector.tensor_copy(dsl, pm[:])

        project_qk(query, NQ, 0, lambda m, j: qT[m][:], gelu=False)
        project_qk(key, N, DIM, lambda m, j: kT[m][j][:], gelu=True)
        wpool_cm.__exit__(None, None, None)

        # ---------------- attention (streaming; V-proj inside head 0) -------
        ones16 = const_pool.tile([1, D], BF16)
        nc.vector.memset(ones16[:], 1.0)

        attn_sb_cm = tc.tile_pool(name="attn_sb", bufs=3)
        attn_sb = attn_sb_cm.__enter__()
        vstage_cm = tc.tile_pool(name="vstage", bufs=3)
        vstage = vstage_cm.__enter__()

        with tc.tile_pool(name="stps", bufs=2, space="PSUM") as stps, \
             tc.tile_pool(name="ptp", bufs=12) as ptp, \
             tc.tile_pool(name="xaps", bufs=2, space="PSUM") as xaps:

            def project_v_tile(t):
                """value tile t -> v_aug[t] (per-head 65-wide, ones col)."""
                vf = vstage.tile([128, DIM], F32, tag="vf")
                nc.sync.dma_start(out=vf[:], in_=value[128 * t:128 * (t + 1), :])
                vb = vstage.tile([128, DIM], BF16, tag="vb")
                nc.vector.tensor_copy(vb[:], vf[:])
                vt = vstage.tile([128, NCT, 128], BF16, tag="vt")
                tpv = stps.tile([128, NCT, 128], BF16, tag="st")
                for c in range(NCT):
                    nc.tensor.transpose(tpv[:, c, :], vb[:, 128 * c:128 * (c + 1)],
                                        ident[:])
                nc.vector.tensor_copy(vt[:], tpv[:])
                pv = stps.tile([128, DIM], F32, tag="st")
                for o0, w in ((0, 512), (512, 256)):
                    for c in range(NCT):
                        nc.tensor.matmul(pv[:, o0:o0 + w], vt[:, c, :],
                                         wv_t[:, c, o0:o0 + w],
                                         start=(c == 0), stop=(c == NCT - 1))
                dst3 = v_aug[t][:].rearrange("p (h w) -> p h w", w=65)
                nc.vector.tensor_copy(dst3[:, :, 0:64],
                                      pv[:].rearrange("p (h w) -> p h w", w=64))
                nc.vector.memset(dst3[:, :, 64:65], 1.0)

            def warm_burst(n_mm):
                """Back-to-back dependency-free matmuls; >=10.24us of gapless
                PE activity makes HAM grant K=8/8 (2.4 GHz)."""
                wt = stps.tile([128, 512], F32, tag="st", name=f"warm{n_mm}")
                for _ in range(n_mm):
                    nc.tensor.matmul(wt[:], ident[:], qT[0][:, 0:512],
                                     start=True, stop=True)
                sink = attn_sb.tile([1, 1], F32, tag="sink")
                nc.vector.tensor_copy(sink[:], wt[0:1, 0:1])

            for mp in range(NCT):
                qTh = [qT[mp][0:64, :], qT[mp][64:128, :]]
                warm_burst(36 if mp == 0 else 14)
                xps = [xaps.tile([65, NQ], F32, tag="xa", name=f"xps0_{mp}"),
                       xaps.tile([65, NQ], F32, tag="xa", name=f"xps1_{mp}")]
                for step in range(2 * NT_K):
                    hh, kt = step % 2, step // 2
                    h = 2 * mp + hh
                    poff = 64 * hh
                    if mp == 0 and hh == 0:
                        project_v_tile(kt)
                    c0 = 128 * (kt % 8)
                    ks = kT[mp][kt // 8][poff:poff + 64, c0:c0 + 128]
                    st = stps.tile([128, NQ], F32, tag="st",
                                   name=f"st{hh}_{mp}_{kt}")
                    for qj in range(2):
                        nc.tensor.matmul(st[:, 512 * qj:512 * (qj + 1)], ks,
                                         qTh[hh][:, 512 * qj:512 * (qj + 1)],
                                         start=True, stop=True)
                    pt = ptp.tile([128, NQ], BF16, tag="pt",
                                  name=f"pt{hh}_{mp}_{kt}")
                    nc.scalar.activation(pt[:], st[:], AF.Exp, scale=SCALE)
                    va = v_aug[kt]
                    for qj in range(2):
                        nc.tensor.matmul(xps[hh][:, 512 * qj:512 * (qj + 1)],
                                         va[:, 65 * h:65 * (h + 1)],
                                         pt[:, 512 * qj:512 * (qj + 1)],
                                         start=(kt == 0), stop=(kt == NT_K - 1),
                                         skip_group_check=True)

                for poff, xp in ((0, xps[0]), (64, xps[1])):
                    d16 = attn_sb.tile([1, NQ], BF16, tag="d16")
                    nc.vector.tensor_copy(d16[:], xp[64:65, :])
                    Rp = stps.tile([D, NQ], F32, tag="st", name=f"Rp{mp}_{poff}")
                    for qj in range(2):
                        nc.tensor.matmul(Rp[:, 512 * qj:512 * (qj + 1)], ones16[:],
                                         d16[:, 512 * qj:512 * (qj + 1)],
                                         start=True, stop=True)
                    Rs = attn_sb.tile([D, NQ], F32, tag="Rs")
                    nc.vector.reciprocal_approx_fast(Rs[:], Rp[:])
                    nc.vector.tensor_tensor(xT[poff:poff + 64, mp, :],
                                            xp[0:64, :], Rs[:], op=ALU.mult)

        vstage_cm.__exit__(None, None, None)
        attn_sb_cm.__exit__(None, None, None)
        wvpool_cm.__exit__(None, None, None)

        # ---------------- output projection ----------------
        with tc.tile_pool(name="ops", bufs=2, space="PSUM") as ops, \
             tc.tile_pool(name="wstage2", bufs=3) as wstage2, \
             tc.tile_pool(name="ostage", bufs=3) as ostage:
            for r in range(NCT):
                wf = wstage2.tile([128, DIM], F32, tag="w2f")
                nc.sync.dma_start(out=wf[:], in_=Wp[128 * r:128 * (r + 1), :])
                wb = wstage2.tile([128, DIM], BF16, tag="w2b")
                nc.vector.tensor_copy(wb[:], wf[:])
                tpw = ops.tile([128, NCT, 128], BF16, tag="w2p")
                for c in range(NCT):
                    nc.tensor.transpose(tpw[:, c, :], wb[:, 128 * c:128 * (c + 1)],
                                        ident[:])
                nc.vector.tensor_copy(wp_t[:, :, 128 * r:128 * (r + 1)], tpw[:])
            for qt in range(NT_Q):
                po = ops.tile([128, DIM], F32)
                for o0, w in ((0, 512), (512, 256)):
                    for c in range(NCT):
                        nc.tensor.matmul(po[:, o0:o0 + w],
                                         xT[:, c, 128 * qt:128 * (qt + 1)],
                                         wp_t[:, c, o0:o0 + w],
                                         start=(c == 0), stop=(c == NCT - 1))
                ot = ostage.tile([128, DIM], F32, tag="ot")
                nc.vector.tensor_tensor(ot[:], po[:], bp_bcast[:], op=ALU.add)
                nc.sync.dma_start(out=out[128 * qt:128 * (qt + 1), :], in_=ot[:])

    nc.compile()
    return nc


_NC_CACHE = {}


def _get_nc():
    if "nc" not in _NC_CACHE:
        _NC_CACHE["nc"] = build_nc()
    return _NC_CACHE["nc"]


def kernel(query, key, value, Wq, Wk, bk, Wv, Wp, bp, _results_hook=None):
    query = np.ascontiguousarray(np.asarray(query, dtype=np.float32))
    key = np.ascontiguousarray(np.asarray(key, dtype=np.float32))
    value = np.ascontiguousarray(np.asarray(value, dtype=np.float32))
    weights = {k: np.ascontiguousarray(np.asarray(v, dtype=np.float32))
               for k, v in (("Wq", Wq), ("Wk", Wk), ("bk", bk),
                            ("Wv", Wv), ("Wp", Wp), ("bp", bp))}

    nc = _get_nc()
    in_maps = []
    for i in range(N_CORES):
        b, qc = i // 4, i % 4
        in_maps.append({
            "query": query[b, NQ * qc:NQ * (qc + 1), :],
            "key": key[b],
            "value": value[b],
            **weights,
        })

    res = bass_utils.run_bass_kernel_spmd(nc, in_maps,
                                          core_ids=list(range(N_CORES)))
    if _results_hook is not None:
        _results_hook(res)

    out = np.empty((B, N, DIM), dtype=np.float32)
    for i in range(N_CORES):
        b, qc = i // 4, i % 4
        out[b, NQ * qc:NQ * (qc + 1), :] = res.results[i]["out"]
    return out

